# revision 39
# baseline (speedup 1.0000x reference)
"""Trainium2 Bass kernel for a single causal attention head.

Problem: x:[8,2048,1024] f32, Wq/Wk/Wv:[64,1024], causal mask.
  Q = x@Wq.T; K = x@Wk.T; V = x@Wv.T
  out = softmax(mask(Q@K.T/sqrt(64))) @ V          -> [8, 2048, 64] f32

Sharding: data-parallel over batch. B == n_cores == 8, so each NeuronCore
computes one full batch element; no collectives.

Per-core pipeline (fp16 matmul inputs, fp32 PSUM accumulation), rebuilt
from the 68 us baseline via trace analysis (now ~55 us).  Structure:

  Weights are packed as P1=[Wq/32;Wq/32] and P2=[Wk;Wv] so that each
  512-row q-block round is gated ONLY by Q: one 8-matmul P1 pass + one
  DVE cast yields Q on both partition halves (scores tile A reads
  partitions 0:64, concurrent tile B reads 64:128 via tile_position).
  The K/V pass (P2), K-hi duplication (PE dup matmul dup64[i,j]=j%64==i),
  V transposes and the vaug copy all run off-gate in proj_rest - only
  the round's DIAGONAL pairs need them, and those come >= 2 pairs in.
  jb0 (whose first pairs are diagonal) instead issues both score banks
  from the A position (kk-lo only, concurrency given up during the
  DMA-paced warmup window) so even round 0 never waits for K-hi.

  Scores per pair of k-tiles: two concurrent 64-contraction matmuls via
  tile_position row tiling into one [128,1024] 2-bank PSUM tile; exp on
  ScalarE (exp(4t), t = s_raw/32 via the host Wq pre-scale; ONE wide op
  per pair unless the diagonal trims make two ops cheaper; the first
  pair is force-split so ACT primes early).  Causal triangle masks
  multiply on GpSimd (it cannot read PSUM; ex lives in SBUF).  AV
  accumulates out_aug[65,q] (row 64 = softmax denominator Z via vaug's
  ones column) per q-block; the host divides by Z (epilogue rescale).

  In-order engine queues drive the emission order: round r emits
  proj(r) -> scores(r,p0,p1) -> AVs(r-1) -> proj_rest(r) ->
  scores(r,p2..) with AVs trailing two pairs behind, so neither a
  DMA-stalled projection nor an ACT-stalled AV ever blocks the exp
  feed (the ScalarE exp stream, ~21.3 us busy, is the pacing engine).

  DMA (two HWDGE queues at ~175 GB/s each when both active; the early
  window ramps slower): weights first on both queues, x0 at 256 KB
  chunks split across queues in consumption order, then 512 KB halves
  per later block; tri/dup deferred past x0.  P1/P2 for jb0 interleave
  per chunk to consume each chunk on arrival.

  HAM clock gate: 40 N=128 identity warmups lift the PE to 8/8 during
  the DMA ramp; N=16 ident fillers (tiny SBUF footprint - wide fillers
  measurably steal SBUF ports from the concurrent x DMA) bridge
  proj(0)'s chunk-wait gaps so the PE never re-throttles mid-kernel.

  Tail: jb3's output cols 0:256 are final one pair early (the last pair
  is tA=256-trimmed) and ship in a separate copy+DMA so the HBM-write
  receipt overlaps the last pair's exp/AV.

  Post-passes on the emitted BIR:
  - _thin_matmul_updates: drop unneeded per-matmul semaphore increments.
  - _split_sync_waits / _patch_tile_drain: pinned-walrus workarounds.
"""

import numpy as np

B, S, E, D = 8, 2048, 1024, 64
NCORES = 8
EC = E // 128   # 8 e-chunks
ST = S // 128   # 16 k-tiles
QB = S // 512   # 4 q-blocks

QSCALE = 1.0 / 32.0   # host pre-scale on Wq; scores psum = s_raw/32, exp scale=4

_cache = {}


def _patch_tile_drain():
    """The pinned walrus rejects >~2 sem waits on one Drain; Tile's tail
    drain waits on every live semaphore.  Split the excess onto standalone
    wait_ge instructions (same semantics: all waits complete before the
    all-engine barrier resets semaphores)."""
    import concourse.mybir as mybir
    import concourse.tile as ctile
    from concourse.vector_clock import ScopedClock

    if getattr(ctile.TileContext, "_drain_patch", False):
        return

    def _drain_and_barrier(self, tick_clock, wait_clock):
        nc = self.nc
        drain_inst = nc.sync.drain()
        wait_clock.add_sem_waits(
            drain_inst.ins, ScopedClock({None: tick_clock.global_clock})
        )
        si = drain_inst.ins.sync_info
        if si is not None and si.on_wait and len(si.on_wait) > 1:
            waits = list(si.on_wait)
            drain_inst.ins.sync_info = mybir.SyncInfo(
                on_wait=[waits[0]], on_update=list(si.on_update)
            )
            handles = {h.num: h for h in self.sems.allocated().values()}
            for w in waits[1:]:
                assert w.wait_mode == "sem-ge-imm", w
                nc.sync.wait_ge(handles[w.id], w.wait_value)
        nc.all_engine_barrier()
        popped = nc._tile_sem_poison_stack.pop()
        assert popped is self._sem_poison
        nc.clear_and_free_semaphores(list(self.sems.allocated().values()))
        nc.all_engine_barrier()

    ctile.TileContext._drain_and_barrier = _drain_and_barrier
    ctile.TileContext._drain_patch = True


def _split_sync_waits(nc, maxw=1):
    """The pinned walrus rejects instructions carrying more than ~2 sem
    waits.  Hoist all but `maxw` waits of every instruction onto dedicated
    NoOps just before it in the same engine stream (engine streams are
    in-order, so semantics are identical)."""
    import concourse.mybir as mybir

    n_new = 0
    for f in nc.m.functions:
        for b in f.blocks:
            out = []
            changed = False
            for inst in b.instructions:
                si = getattr(inst, "sync_info", None)
                if si is not None and si.on_wait and len(si.on_wait) > maxw:
                    waits = list(si.on_wait)
                    extras, keep = waits[:-maxw], waits[-maxw:]
                    for k, w in enumerate(extras):
                        nop = mybir.InstNoOp(
                            name=f"{inst.name}-hw{k}", ins=[], outs=[],
                            sync_info=mybir.SyncInfo(on_wait=[w], on_update=[]),
                        )
                        nop.engine = inst.engine
                        nc.register_instruction(nop)
                        out.append(nop)
                        n_new += 1
                    inst.sync_info = mybir.SyncInfo(
                        on_wait=keep, on_update=list(si.on_update)
                    )
                    changed = True
                out.append(inst)
            if changed:
                b.instructions = out
    return n_new


def _thin_matmul_updates(nc):
    """Tile puts a progress-semaphore increment on EVERY matmul; the EVT_SEM
    write forces each matmul to fully drain before the next issues (~465 ns
    cadence for N=512 instead of ~215).  Keep increments only on matmuls some
    instruction actually waits for, and remap every wait value on that
    semaphore to the new (smaller) increment counts.  PE completes in pc
    order, so dropping an unneeded increment never reorders anything."""
    import concourse.mybir as mybir
    import bisect

    insts = [i for f in nc.m.functions for b in f.blocks for i in b.instructions]
    # map: sem id -> ordered list of matmul instructions updating it
    upd = {}
    for i in insts:
        si = getattr(i, "sync_info", None)
        if si is None or not isinstance(i, mybir.InstMatmult):
            continue
        for u in si.on_update:
            upd.setdefault(u.id, []).append(i)
    for sem_id, updaters in upd.items():
        waits = []
        for i in insts:
            si = getattr(i, "sync_info", None)
            if si is None:
                continue
            for w in si.on_wait:
                if w.id == sem_id and w.wait_mode == "sem-ge-imm":
                    waits.append(w)
        if not waits:
            continue
        needed = set()
        for w in waits:
            v = w.wait_value
            if 1 <= v <= len(updaters):
                needed.add(v - 1)   # index of the v-th incrementer
            else:
                needed.add(len(updaters) - 1)
        # keep exactly the increments some wait targets (matmuls complete in
        # pc order, so every waiter still waits on the same matmul)
        keep = []
        for idx, i in enumerate(updaters):
            if idx in needed:
                keep.append(idx)
            else:
                si = i.sync_info
                i.sync_info = mybir.SyncInfo(
                    on_wait=list(si.on_wait),
                    on_update=[u for u in si.on_update if u.id != sem_id],
                )
        # remap wait values: new value = #kept among first v updaters,
        # rounded up to include the next kept one if the v-th was dropped
        for w in waits:
            v = min(max(w.wait_value, 1), len(updaters))
            tgt = v - 1
            pos = bisect.bisect_left(keep, tgt)
            assert pos < len(keep), (sem_id, v, keep[-5:])
            w.wait_value = pos + 1


def _build_nc():
    import concourse.bass as bass
    import concourse.mybir as mybir
    from concourse import tile
    from concourse.masks import make_identity

    _patch_tile_drain()

    fp16 = mybir.dt.float16
    f32 = mybir.dt.float32
    EXP = mybir.ActivationFunctionType.Exp

    nc = bass.Bass("TRN2", target_bir_lowering=False)
    # xh[p, jb, ec*512+c] = x[b][jb*512+c, ec*128+p]; 256KB chunk DMAs
    xh_d = nc.dram_tensor("xh", [128, QB, EC * 512], fp16, kind="ExternalInput")
    # wconst[p] = wqv chunks | wkk chunks | trimask | dup  (three DMAs)
    wc_d = nc.dram_tensor("wconst", [128, 2 * E + 256], fp16, kind="ExternalInput")
    out_d = nc.dram_tensor("out", [D + 1, S], f32, kind="ExternalOutput")

    with tile.TileContext(nc) as tc:
        with (
            tc.tile_pool(name="singles", bufs=1) as singles,
            tc.tile_pool(name="sb", bufs=2) as sb,
            tc.tile_pool(name="expp", bufs=16) as expp,
            tc.tile_pool(name="psA", bufs=3, space="PSUM") as psA,
            tc.tile_pool(name="psS", bufs=2, space="PSUM") as psS,
            tc.tile_pool(name="psO", bufs=1, space="PSUM") as psO,
        ):
            # ---- constants / inputs ----
            ident = singles.tile([128, 128], fp16)
            make_identity(nc, ident[:])
            wcst = singles.tile([128, 2 * E + 256], fp16)
            xin = singles.tile([128, QB, EC * 512], fp16)
            # x0 entirely on the sync queue (the scalar queue drains slowly
            # in the early window); weights lead their consumers; the rest
            # split across queues in consumption order.
            nc.sync.dma_start(wcst[:, 0:E], wc_d[:, 0:E])              # wqq
            nc.scalar.dma_start(wcst[:, E:2 * E], wc_d[:, E:2 * E])    # wkv
            nc.sync.dma_start(xin[:, 0, 0:1024], xh_d[:, 0, 0:1024])
            nc.scalar.dma_start(xin[:, 0, 1024:2048], xh_d[:, 0, 1024:2048])
            nc.sync.dma_start(xin[:, 0, 2048:3072], xh_d[:, 0, 2048:3072])
            nc.scalar.dma_start(xin[:, 0, 3072:4096], xh_d[:, 0, 3072:4096])
            nc.scalar.dma_start(wcst[:, 2 * E:], wc_d[:, 2 * E:])      # tri+dup
            for jb in range(1, QB):
                nc.sync.dma_start(xin[:, jb, 0:2048], xh_d[:, jb, 0:2048])
                nc.scalar.dma_start(xin[:, jb, 2048:4096], xh_d[:, jb, 2048:4096])
            wqq = wcst[:, 0:E].rearrange("p (ec c) -> p ec c", ec=EC)
            wkv = wcst[:, E:2 * E].rearrange("p (ec c) -> p ec c", ec=EC)
            trimask = wcst[:, 2 * E:2 * E + 128]
            dup64 = wcst[0:64, 2 * E + 128:2 * E + 256]

            qv = singles.tile([128, S], fp16)    # rows 64:128 VT (V-hi)
            qq = singles.tile([128, S], fp16)    # QT/32 on BOTH halves
            kk = singles.tile([128, S], fp16)    # KT on both halves
            vaug = singles.tile([128, ST, 65], fp16)
            nc.vector.memset(vaug[:, :, 64:65], 1.0)

            # ---- PE warmup: lift HAM to 8/8 during the initial DMA.
            # N=16 matmuls: enough PE-busy for the HAM activity window but
            # ~2KB SBUF reads each, so the warmup does not steal SBUF port
            # bandwidth from the concurrent x DMA stream (N=128 idents
            # measured to slow the early DMA window by ~2.5us).
            ps_fill = psS.tile([128, 1024], f32, tag="s", name="ps_fill")
            for i in range(40):
                nc.tensor.matmul(ps_fill[:, 0:128], ident[:], ident[:],
                                 start=True, stop=True)

            def pe_filler(n):
                # N=16 fillers: keep the HAM busy-window alive through
                # DMA-paced stretches without stealing SBUF bandwidth
                for i in range(n):
                    nc.tensor.matmul(ps_fill[0:16, 128:144], ident[:, 0:16],
                                     ident[:, 0:16], start=True, stop=True)

            proj_ps = {}

            def proj(jb):
                """The round gate is Q only: P1=[Wq/32;Wq/32] produces Q on
                BOTH partition halves in one 8-matmul pass, one cast.  For
                jb0 (whose first pairs are diagonal and also need K) P2 is
                interleaved per chunk; otherwise P2 waits for proj_rest."""
                qs = slice(jb * 512, (jb + 1) * 512)
                ps1 = psA.tile([128, 512], f32, tag="p", name=f"ps1_{jb}")
                ps2 = psA.tile([128, 512], f32, tag="p", name=f"ps2_{jb}")
                proj_ps[jb] = (ps1, ps2)
                for ec in range(EC):
                    if jb == 0 and ec % 2 == 0:
                        pe_filler(10)   # bridge the chunk-DMA gaps (HAM)
                    nc.tensor.matmul(ps1[:], wqq[:, ec, :], xin[:, jb, ec * 512:(ec + 1) * 512],
                                     start=(ec == 0), stop=(ec == EC - 1))
                    if jb == 0:
                        nc.tensor.matmul(ps2[:], wkv[:, ec, :], xin[:, jb, ec * 512:(ec + 1) * 512],
                                         start=(ec == 0), stop=(ec == EC - 1))
                if jb == 0:   # jb0's diagonal pairs need K-lo (only) too
                    nc.vector.tensor_copy(kk[0:64, qs], ps2[0:64, :])
                nc.vector.tensor_copy(qq[:, qs], ps1[:])                 # Q both

            def proj_rest(jb):
                """Off-gate remainder: P2=[Wk;Wv] pass (jb>0), K-lo cast,
                K-hi via PE dup matmul (dup64[i,j] = j%64==i), V-hi cast,
                V transposes + vaug."""
                qs = slice(jb * 512, (jb + 1) * 512)
                ps1, ps2 = proj_ps.pop(jb)
                if jb != 0:
                    for ec in range(EC):
                        nc.tensor.matmul(ps2[:], wkv[:, ec, :], xin[:, jb, ec * 512:(ec + 1) * 512],
                                         start=(ec == 0), stop=(ec == EC - 1))
                    nc.vector.tensor_copy(kk[0:64, qs], ps2[0:64, :])
                ps_k2 = psA.tile([128, 512], f32, tag="p", name=f"ps_k2_{jb}")
                nc.tensor.matmul(ps_k2[:], dup64, kk[0:64, qs],
                                 start=True, stop=True)
                nc.vector.tensor_copy(kk[64:128, qs], ps_k2[64:128, :])
                nc.vector.tensor_copy(qv[64:128, qs], ps2[64:128, :])    # V-hi
                ps_tr = psA.tile([128, 4, 64], fp16, tag="p", name=f"ps_tr{jb}")
                for t in range(4):
                    si = jb * 4 + t
                    nc.tensor.transpose(
                        ps_tr[:, t, :], qv[64:128, si * 128:(si + 1) * 128],
                        ident[64:128, 64:128])
                nc.vector.tensor_copy(vaug[:, jb * 4:jb * 4 + 4, 0:64], ps_tr[:])

            def norm_finish(jb, lo=0, hi=512):
                # ship unnormalized outT + Z; host divides (epilogue rescale)
                qs0 = jb * 512
                ou = norm_ou.setdefault(
                    jb, sb.tile([65, 512], f32, tag="o", name=f"ou{jb}"))
                nc.vector.tensor_copy(ou[:, lo:hi], norm_ps[jb][:, lo:hi])
                nc.sync.dma_start(out_d[:, qs0 + lo:qs0 + hi], ou[:, lo:hi])

            norm_ou = {}

            norm_ps = {}
            exp_n = [0]

            def emit_exp(ex, ps_s, tA, tB, rA, rB):
                """exp of one pair: banks [tA:512] and [512+tB:1024] on
                ScalarE (the only exp-capable engine; the pinned walrus
                rejects InstCustomDveAnt so no DVE offload).  Triangle
                masks on GpSimd (it cannot read PSUM but ex is SBUF)."""
                if tA + tB <= 352 and exp_n[0] > 0:   # one wide op is cheaper
                    nc.scalar.activation(ex[:, tA:1024], ps_s[:, tA:1024],
                                         EXP, scale=4.0)
                else:
                    nc.scalar.activation(ex[:, tA:512], ps_s[:, tA:512],
                                         EXP, scale=4.0)
                    nc.scalar.activation(ex[:, 512 + tB:1024],
                                         ps_s[:, 512 + tB:1024],
                                         EXP, scale=4.0)
                exp_n[0] += 1
                if rA >= 0:
                    nc.gpsimd.tensor_mul(ex[:, tA:tA + 128],
                                         ex[:, tA:tA + 128], trimask)
                if rB >= 0:
                    nc.gpsimd.tensor_mul(ex[:, 512 + tB:512 + tB + 128],
                                         ex[:, 512 + tB:512 + tB + 128],
                                         trimask)

            ex_of = {}   # (jb, p) -> ex tile
            pend = []    # (jb, p) AVs not yet emitted, in order

            def emit_scores(jb, p):
                qs0 = jb * 512
                kiA, kiB = 2 * p, 2 * p + 1
                rA, rB = kiA - 4 * jb, kiB - 4 * jb   # >=0 -> diagonal
                tA = 128 * max(rA, 0)                 # column trim
                tB = 128 * max(rB, 0)
                ps_s = psS.tile([128, 1024], f32, tag="s", name=f"s{jb}_{p}")
                nc.tensor.matmul(
                    ps_s[:, tA:512], kk[0:64, kiA * 128:(kiA + 1) * 128],
                    qq[0:64, qs0 + tA:qs0 + 512], start=True, stop=True)
                if jb == 0:
                    # both banks from the A position: no kk-hi / Q-hi on
                    # the very first round's critical path (PE is DMA-
                    # stalled here anyway, concurrency is free to give up)
                    nc.tensor.matmul(
                        ps_s[:, 512 + tB:1024], kk[0:64, kiB * 128:(kiB + 1) * 128],
                        qq[0:64, qs0 + tB:qs0 + 512], start=True, stop=True)
                else:
                    nc.tensor.matmul(
                        ps_s[:, 512 + tB:1024], kk[64:128, kiB * 128:(kiB + 1) * 128],
                        qq[64:128, qs0 + tB:qs0 + 512], start=True, stop=True,
                        tile_position=(64, 0))
                ex = expp.tile([128, 1024], fp16, tag="ex", name=f"ex{jb}_{p}")
                ex_of[(jb, p)] = ex
                emit_exp(ex, ps_s, tA, tB, rA, rB)
                pend.append((jb, p))

            def emit_av(jb, p):
                npair = 2 * jb + 2
                last = 2 * npair - 1
                kiA, kiB = 2 * p, 2 * p + 1
                tA = 128 * max(kiA - 4 * jb, 0)
                tB = 128 * max(kiB - 4 * jb, 0)
                if p == 0:
                    norm_ps[jb] = psO.tile([65, 512], f32, tag="o",
                                           name=f"ps_o{jb}")
                ps_o = norm_ps[jb]
                ex = ex_of.pop((jb, p))
                nc.tensor.matmul(ps_o[:, tA:512], vaug[:, kiA, :],
                                 ex[:, tA:512], start=(2 * p == 0),
                                 stop=(2 * p == last))
                nc.tensor.matmul(ps_o[:, tB:512], vaug[:, kiB, :],
                                 ex[:, 512 + tB:1024],
                                 start=(2 * p + 1 == 0),
                                 stop=(2 * p + 1 == last))

            def flush_avs(upto_jb, finish=True):
                while pend and pend[0][0] <= upto_jb:
                    jb, p = pend.pop(0)
                    emit_av(jb, p)
                    if p == 2 * jb + 1 and finish:
                        norm_finish(jb)

            # round structure: proj(r) and the new round's first scores go
            # ahead of the previous round's AVs in the in-order PE stream,
            # so neither DMA-stalled projections nor ACT-stalled AVs ever
            # block the exp feed.
            for r in range(QB):
                proj(r)
                if r == 0:
                    proj_rest(0)       # jb0's first pairs are diagonal
                    emit_scores(0, 0)
                    emit_scores(0, 1)
                    continue
                emit_scores(r, 0)
                emit_scores(r, 1)
                flush_avs(r - 1)
                proj_rest(r)
                npair = 2 * r + 2
                for p in range(2, npair):
                    emit_scores(r, p)
                    # trail this round's AVs two pairs behind the scores
                    while len(pend) > 2:
                        jb_, p_ = pend.pop(0)
                        emit_av(jb_, p_)
                        if p_ == 2 * jb_ + 1:
                            norm_finish(jb_)
            # endgame: remaining jb3 AVs, first output half shipped before
            # the last (tA=256-trimmed) pair so the HBM-write receipt
            # overlaps it.
            while pend:
                jb_, p_ = pend.pop(0)
                emit_av(jb_, p_)
                if jb_ == QB - 1 and p_ == 2 * jb_ and len(pend) == 1:
                    norm_finish(jb_, 0, 256)
            norm_finish(QB - 1, 256, 512)

    _thin_matmul_updates(nc)
    _split_sync_waits(nc)
    nc.finalize()
    return nc


def kernel(x, Wq, Wk, Wv, attention_mask=None, **_unused):
    from concourse.bass_utils import run_bass_kernel_spmd

    if "nc" not in _cache:
        _cache["nc"] = _build_nc()
    nc = _cache["nc"]

    def chunked(wT):   # [E, 128] -> [128, E] partition-major chunk layout
        return wT.reshape(EC, 128, 128).transpose(1, 0, 2).reshape(128, E)

    wqqT = np.concatenate([np.asarray(Wq) * QSCALE, np.asarray(Wq) * QSCALE], 0).T.astype(np.float16)
    wkvT = np.concatenate([np.asarray(Wk), np.asarray(Wv)], 0).T.astype(np.float16)
    tri = (np.arange(128)[:, None] <= np.arange(128)[None, :]).astype(np.float16)
    dup = np.zeros((128, 128), np.float16)
    dup[np.arange(128) % 64, np.arange(128)] = 1.0
    wconst = np.ascontiguousarray(
        np.concatenate([chunked(wqqT), chunked(wkvT), tri, dup], 1))
    x = np.asarray(x)
    in_maps = [
        {
            "xh": np.ascontiguousarray(
                x[b].T.astype(np.float16).reshape(EC, 128, QB, 512)
                .transpose(1, 2, 0, 3).reshape(128, QB, EC * 512)),
            "wconst": wconst,
        }
        for b in range(B)
    ]
    import os

    tmpdir = None
    if os.environ.get("BASS_TRACE"):
        tmpdir = os.environ.get("BASS_TRACE_DIR", "/tmp/bass_trace")
        os.makedirs(tmpdir, exist_ok=True)
    res = run_bass_kernel_spmd(nc, in_maps, core_ids=list(range(NCORES)), tmpdir=tmpdir)
    out = np.stack(
        [(res.results[b]["out"][0:D] / res.results[b]["out"][D:D + 1]).T
         for b in range(B)], 0)
    _cache["last_exec_time_ns"] = res.exec_time_ns
    _cache["trace_dir"] = tmpdir
    return out.astype(np.float32)


# revision 40
# speedup vs baseline: 1.0080x; 1.0080x over previous
"""Trainium2 Bass kernel for a single causal attention head.

Problem: x:[8,2048,1024] f32, Wq/Wk/Wv:[64,1024], causal mask.
  Q = x@Wq.T; K = x@Wk.T; V = x@Wv.T
  out = softmax(mask(Q@K.T/sqrt(64))) @ V          -> [8, 2048, 64] f32

Sharding: data-parallel over batch. B == n_cores == 8, so each NeuronCore
computes one full batch element; no collectives.

Per-core pipeline (fp16 matmul inputs, fp32 PSUM accumulation), rebuilt
from the 68 us baseline via trace analysis (now ~55 us).  Structure:

  Weights are packed as P1=[Wq/32;Wq/32] and P2=[Wk;Wv] so that each
  512-row q-block round is gated ONLY by Q: one 8-matmul P1 pass + one
  DVE cast yields Q on both partition halves (scores tile A reads
  partitions 0:64, concurrent tile B reads 64:128 via tile_position).
  The K/V pass (P2), K-hi duplication (PE dup matmul dup64[i,j]=j%64==i),
  V transposes and the vaug copy all run off-gate in proj_rest - only
  the round's DIAGONAL pairs need them, and those come >= 2 pairs in.
  jb0 (whose first pairs are diagonal) instead issues both score banks
  from the A position (kk-lo only, concurrency given up during the
  DMA-paced warmup window) so even round 0 never waits for K-hi.

  Scores per pair of k-tiles: two concurrent 64-contraction matmuls via
  tile_position row tiling into one [128,1024] 2-bank PSUM tile; exp on
  ScalarE (exp(4t), t = s_raw/32 via the host Wq pre-scale; ONE wide op
  per pair unless the diagonal trims make two ops cheaper; the first
  pair is force-split so ACT primes early).  Causal triangle masks
  multiply on GpSimd (it cannot read PSUM; ex lives in SBUF).  AV
  accumulates out_aug[65,q] (row 64 = softmax denominator Z via vaug's
  ones column) per q-block; the host divides by Z (epilogue rescale).

  In-order engine queues drive the emission order: round r emits
  proj(r) -> scores(r,p0,p1) -> AVs(r-1) -> proj_rest(r) ->
  scores(r,p2..) with AVs trailing two pairs behind, so neither a
  DMA-stalled projection nor an ACT-stalled AV ever blocks the exp
  feed (the ScalarE exp stream, ~21.3 us busy, is the pacing engine).

  DMA (two HWDGE queues at ~175 GB/s each when both active; the early
  window ramps slower): weights first on both queues, x0 at 256 KB
  chunks split across queues in consumption order, then 512 KB halves
  per later block; tri/dup deferred past x0.  P1/P2 for jb0 interleave
  per chunk to consume each chunk on arrival.

  HAM clock gate: 40 N=128 identity warmups lift the PE to 8/8 during
  the DMA ramp; N=16 ident fillers (tiny SBUF footprint - wide fillers
  measurably steal SBUF ports from the concurrent x DMA) bridge
  proj(0)'s chunk-wait gaps so the PE never re-throttles mid-kernel.

  Tail: jb3's output cols 0:256 are final one pair early (the last pair
  is tA=256-trimmed) and ship in a separate copy+DMA so the HBM-write
  receipt overlaps the last pair's exp/AV.

  Post-passes on the emitted BIR:
  - _thin_matmul_updates: drop unneeded per-matmul semaphore increments.
  - _split_sync_waits / _patch_tile_drain: pinned-walrus workarounds.
"""

import numpy as np

B, S, E, D = 8, 2048, 1024, 64
NCORES = 8
EC = E // 128   # 8 e-chunks
ST = S // 128   # 16 k-tiles
QB = S // 512   # 4 q-blocks

QSCALE = 1.0 / 32.0   # host pre-scale on Wq; scores psum = s_raw/32, exp scale=4

_cache = {}


def _patch_tile_drain():
    """The pinned walrus rejects >~2 sem waits on one Drain; Tile's tail
    drain waits on every live semaphore.  Split the excess onto standalone
    wait_ge instructions (same semantics: all waits complete before the
    all-engine barrier resets semaphores)."""
    import concourse.mybir as mybir
    import concourse.tile as ctile
    from concourse.vector_clock import ScopedClock

    if getattr(ctile.TileContext, "_drain_patch", False):
        return

    def _drain_and_barrier(self, tick_clock, wait_clock):
        nc = self.nc
        drain_inst = nc.sync.drain()
        wait_clock.add_sem_waits(
            drain_inst.ins, ScopedClock({None: tick_clock.global_clock})
        )
        si = drain_inst.ins.sync_info
        if si is not None and si.on_wait and len(si.on_wait) > 1:
            waits = list(si.on_wait)
            drain_inst.ins.sync_info = mybir.SyncInfo(
                on_wait=[waits[0]], on_update=list(si.on_update)
            )
            handles = {h.num: h for h in self.sems.allocated().values()}
            for w in waits[1:]:
                assert w.wait_mode == "sem-ge-imm", w
                nc.sync.wait_ge(handles[w.id], w.wait_value)
        nc.all_engine_barrier()
        popped = nc._tile_sem_poison_stack.pop()
        assert popped is self._sem_poison
        nc.clear_and_free_semaphores(list(self.sems.allocated().values()))
        nc.all_engine_barrier()

    ctile.TileContext._drain_and_barrier = _drain_and_barrier
    ctile.TileContext._drain_patch = True


def _split_sync_waits(nc, maxw=1):
    """The pinned walrus rejects instructions carrying more than ~2 sem
    waits.  Hoist all but `maxw` waits of every instruction onto dedicated
    NoOps just before it in the same engine stream (engine streams are
    in-order, so semantics are identical)."""
    import concourse.mybir as mybir

    n_new = 0
    for f in nc.m.functions:
        for b in f.blocks:
            out = []
            changed = False
            for inst in b.instructions:
                si = getattr(inst, "sync_info", None)
                if si is not None and si.on_wait and len(si.on_wait) > maxw:
                    waits = list(si.on_wait)
                    extras, keep = waits[:-maxw], waits[-maxw:]
                    for k, w in enumerate(extras):
                        nop = mybir.InstNoOp(
                            name=f"{inst.name}-hw{k}", ins=[], outs=[],
                            sync_info=mybir.SyncInfo(on_wait=[w], on_update=[]),
                        )
                        nop.engine = inst.engine
                        nc.register_instruction(nop)
                        out.append(nop)
                        n_new += 1
                    inst.sync_info = mybir.SyncInfo(
                        on_wait=keep, on_update=list(si.on_update)
                    )
                    changed = True
                out.append(inst)
            if changed:
                b.instructions = out
    return n_new


def _thin_matmul_updates(nc):
    """Tile puts a progress-semaphore increment on EVERY matmul; the EVT_SEM
    write forces each matmul to fully drain before the next issues (~465 ns
    cadence for N=512 instead of ~215).  Keep increments only on matmuls some
    instruction actually waits for, and remap every wait value on that
    semaphore to the new (smaller) increment counts.  PE completes in pc
    order, so dropping an unneeded increment never reorders anything."""
    import concourse.mybir as mybir
    import bisect

    insts = [i for f in nc.m.functions for b in f.blocks for i in b.instructions]
    # map: sem id -> ordered list of matmul instructions updating it
    upd = {}
    for i in insts:
        si = getattr(i, "sync_info", None)
        if si is None or not isinstance(i, mybir.InstMatmult):
            continue
        for u in si.on_update:
            upd.setdefault(u.id, []).append(i)
    for sem_id, updaters in upd.items():
        waits = []
        for i in insts:
            si = getattr(i, "sync_info", None)
            if si is None:
                continue
            for w in si.on_wait:
                if w.id == sem_id and w.wait_mode == "sem-ge-imm":
                    waits.append(w)
        if not waits:
            continue
        needed = set()
        for w in waits:
            v = w.wait_value
            if 1 <= v <= len(updaters):
                needed.add(v - 1)   # index of the v-th incrementer
            else:
                needed.add(len(updaters) - 1)
        # keep exactly the increments some wait targets (matmuls complete in
        # pc order, so every waiter still waits on the same matmul)
        keep = []
        for idx, i in enumerate(updaters):
            if idx in needed:
                keep.append(idx)
            else:
                si = i.sync_info
                i.sync_info = mybir.SyncInfo(
                    on_wait=list(si.on_wait),
                    on_update=[u for u in si.on_update if u.id != sem_id],
                )
        # remap wait values: new value = #kept among first v updaters,
        # rounded up to include the next kept one if the v-th was dropped
        for w in waits:
            v = min(max(w.wait_value, 1), len(updaters))
            tgt = v - 1
            pos = bisect.bisect_left(keep, tgt)
            assert pos < len(keep), (sem_id, v, keep[-5:])
            w.wait_value = pos + 1


def _build_nc():
    import concourse.bass as bass
    import concourse.mybir as mybir
    from concourse import tile
    from concourse.masks import make_identity

    _patch_tile_drain()

    fp16 = mybir.dt.float16
    f32 = mybir.dt.float32
    EXP = mybir.ActivationFunctionType.Exp

    nc = bass.Bass("TRN2", target_bir_lowering=False)
    # xh[p, jb, ec*512+c] = x[b][jb*512+c, ec*128+p]; 256KB chunk DMAs
    xh_d = nc.dram_tensor("xh", [128, QB, EC * 512], fp16, kind="ExternalInput")
    # wconst[p] = wqv chunks | wkk chunks | trimask | dup  (three DMAs)
    wc_d = nc.dram_tensor("wconst", [128, 2 * E + 256], fp16, kind="ExternalInput")
    out_d = nc.dram_tensor("out", [D + 1, S], f32, kind="ExternalOutput")

    with tile.TileContext(nc) as tc:
        with (
            tc.tile_pool(name="singles", bufs=1) as singles,
            tc.tile_pool(name="sb", bufs=2) as sb,
            tc.tile_pool(name="expp", bufs=16) as expp,
            tc.tile_pool(name="psA", bufs=3, space="PSUM") as psA,
            tc.tile_pool(name="psS", bufs=2, space="PSUM") as psS,
            tc.tile_pool(name="psO", bufs=1, space="PSUM") as psO,
        ):
            # ---- constants / inputs ----
            ident = singles.tile([128, 128], fp16)
            make_identity(nc, ident[:])
            wcst = singles.tile([128, 2 * E + 256], fp16)
            xin = singles.tile([128, QB, EC * 512], fp16)
            # x0 entirely on the sync queue (the scalar queue drains slowly
            # in the early window); weights lead their consumers; the rest
            # split across queues in consumption order.
            nc.sync.dma_start(wcst[:, 0:E], wc_d[:, 0:E])              # wqq
            nc.sync.dma_start(wcst[:, E:2 * E], wc_d[:, E:2 * E])      # wkv
            nc.scalar.dma_start(xin[:, 0, 1024:2048], xh_d[:, 0, 1024:2048])
            nc.scalar.dma_start(xin[:, 0, 3072:4096], xh_d[:, 0, 3072:4096])
            nc.sync.dma_start(xin[:, 0, 0:1024], xh_d[:, 0, 0:1024])
            nc.sync.dma_start(xin[:, 0, 2048:3072], xh_d[:, 0, 2048:3072])
            nc.scalar.dma_start(wcst[:, 2 * E:], wc_d[:, 2 * E:])      # tri+dup
            for jb in range(1, QB):
                nc.sync.dma_start(xin[:, jb, 0:2048], xh_d[:, jb, 0:2048])
                nc.scalar.dma_start(xin[:, jb, 2048:4096], xh_d[:, jb, 2048:4096])
            wqq = wcst[:, 0:E].rearrange("p (ec c) -> p ec c", ec=EC)
            wkv = wcst[:, E:2 * E].rearrange("p (ec c) -> p ec c", ec=EC)
            trimask = wcst[:, 2 * E:2 * E + 128]
            dup64 = wcst[0:64, 2 * E + 128:2 * E + 256]

            qv = singles.tile([128, S], fp16)    # rows 64:128 VT (V-hi)
            qq = singles.tile([128, S], fp16)    # QT/32 on BOTH halves
            kk = singles.tile([128, S], fp16)    # KT on both halves
            vaug = singles.tile([128, ST, 65], fp16)
            nc.vector.memset(vaug[:, :, 64:65], 1.0)

            # ---- PE warmup: lift HAM to 8/8 during the initial DMA.
            # N=16 matmuls: enough PE-busy for the HAM activity window but
            # ~2KB SBUF reads each, so the warmup does not steal SBUF port
            # bandwidth from the concurrent x DMA stream (N=128 idents
            # measured to slow the early DMA window by ~2.5us).
            ps_fill = psS.tile([128, 1024], f32, tag="s", name="ps_fill")
            for i in range(40):
                nc.tensor.matmul(ps_fill[:, 0:128], ident[:], ident[:],
                                 start=True, stop=True)

            def pe_filler(n):
                # N=16 fillers: keep the HAM busy-window alive through
                # DMA-paced stretches without stealing SBUF bandwidth
                for i in range(n):
                    nc.tensor.matmul(ps_fill[0:16, 128:144], ident[:, 0:16],
                                     ident[:, 0:16], start=True, stop=True)

            proj_ps = {}

            def proj(jb):
                """The round gate is Q only: P1=[Wq/32;Wq/32] produces Q on
                BOTH partition halves in one 8-matmul pass, one cast.  For
                jb0 (whose first pairs are diagonal and also need K) P2 is
                interleaved per chunk; otherwise P2 waits for proj_rest."""
                qs = slice(jb * 512, (jb + 1) * 512)
                ps1 = psA.tile([128, 512], f32, tag="p", name=f"ps1_{jb}")
                ps2 = psA.tile([128, 512], f32, tag="p", name=f"ps2_{jb}")
                proj_ps[jb] = (ps1, ps2)
                for ec in range(EC):
                    if jb == 0 and ec % 2 == 0:
                        pe_filler(10)   # bridge the chunk-DMA gaps (HAM)
                    nc.tensor.matmul(ps1[:], wqq[:, ec, :], xin[:, jb, ec * 512:(ec + 1) * 512],
                                     start=(ec == 0), stop=(ec == EC - 1))
                    if jb == 0:
                        nc.tensor.matmul(ps2[:], wkv[:, ec, :], xin[:, jb, ec * 512:(ec + 1) * 512],
                                         start=(ec == 0), stop=(ec == EC - 1))
                if jb == 0:   # jb0's diagonal pairs need K-lo (only) too
                    nc.vector.tensor_copy(kk[0:64, qs], ps2[0:64, :])
                    # jb0's B banks run from the A position: Q-hi unused
                    nc.vector.tensor_copy(qq[0:64, qs], ps1[0:64, :])
                else:
                    nc.vector.tensor_copy(qq[:, qs], ps1[:])             # Q both

            def proj_rest(jb):
                """Off-gate remainder: P2=[Wk;Wv] pass (jb>0), K-lo cast,
                K-hi via PE dup matmul (dup64[i,j] = j%64==i), V-hi cast,
                V transposes + vaug."""
                qs = slice(jb * 512, (jb + 1) * 512)
                ps1, ps2 = proj_ps.pop(jb)
                if jb != 0:
                    for ec in range(EC):
                        nc.tensor.matmul(ps2[:], wkv[:, ec, :], xin[:, jb, ec * 512:(ec + 1) * 512],
                                         start=(ec == 0), stop=(ec == EC - 1))
                    nc.vector.tensor_copy(kk[0:64, qs], ps2[0:64, :])
                ps_k2 = psA.tile([128, 512], f32, tag="p", name=f"ps_k2_{jb}")
                nc.tensor.matmul(ps_k2[:], dup64, kk[0:64, qs],
                                 start=True, stop=True)
                nc.vector.tensor_copy(kk[64:128, qs], ps_k2[64:128, :])
                nc.vector.tensor_copy(qv[64:128, qs], ps2[64:128, :])    # V-hi
                ps_tr = psA.tile([128, 4, 64], fp16, tag="p", name=f"ps_tr{jb}")
                for t in range(4):
                    si = jb * 4 + t
                    nc.tensor.transpose(
                        ps_tr[:, t, :], qv[64:128, si * 128:(si + 1) * 128],
                        ident[64:128, 64:128])
                nc.vector.tensor_copy(vaug[:, jb * 4:jb * 4 + 4, 0:64], ps_tr[:])

            def norm_finish(jb, lo=0, hi=512):
                # ship unnormalized outT + Z; host divides (epilogue rescale)
                qs0 = jb * 512
                ou = norm_ou.setdefault(
                    jb, sb.tile([65, 512], f32, tag="o", name=f"ou{jb}"))
                nc.vector.tensor_copy(ou[:, lo:hi], norm_ps[jb][:, lo:hi])
                nc.sync.dma_start(out_d[:, qs0 + lo:qs0 + hi], ou[:, lo:hi])

            norm_ou = {}

            norm_ps = {}
            exp_n = [0]

            def emit_exp(ex, ps_s, tA, tB, rA, rB):
                """exp of one pair: banks [tA:512] and [512+tB:1024] on
                ScalarE (the only exp-capable engine; the pinned walrus
                rejects InstCustomDveAnt so no DVE offload).  Triangle
                masks on GpSimd (it cannot read PSUM but ex is SBUF)."""
                if tA + tB <= 352 and exp_n[0] > 0:   # one wide op is cheaper
                    nc.scalar.activation(ex[:, tA:1024], ps_s[:, tA:1024],
                                         EXP, scale=4.0)
                else:
                    nc.scalar.activation(ex[:, tA:512], ps_s[:, tA:512],
                                         EXP, scale=4.0)
                    nc.scalar.activation(ex[:, 512 + tB:1024],
                                         ps_s[:, 512 + tB:1024],
                                         EXP, scale=4.0)
                exp_n[0] += 1
                if rA >= 0:
                    nc.gpsimd.tensor_mul(ex[:, tA:tA + 128],
                                         ex[:, tA:tA + 128], trimask)
                if rB >= 0:
                    nc.gpsimd.tensor_mul(ex[:, 512 + tB:512 + tB + 128],
                                         ex[:, 512 + tB:512 + tB + 128],
                                         trimask)

            ex_of = {}   # (jb, p) -> ex tile
            pend = []    # (jb, p) AVs not yet emitted, in order

            def emit_scores(jb, p):
                qs0 = jb * 512
                kiA, kiB = 2 * p, 2 * p + 1
                rA, rB = kiA - 4 * jb, kiB - 4 * jb   # >=0 -> diagonal
                tA = 128 * max(rA, 0)                 # column trim
                tB = 128 * max(rB, 0)
                ps_s = psS.tile([128, 1024], f32, tag="s", name=f"s{jb}_{p}")
                nc.tensor.matmul(
                    ps_s[:, tA:512], kk[0:64, kiA * 128:(kiA + 1) * 128],
                    qq[0:64, qs0 + tA:qs0 + 512], start=True, stop=True)
                if jb == 0:
                    # both banks from the A position: no kk-hi / Q-hi on
                    # the very first round's critical path (PE is DMA-
                    # stalled here anyway, concurrency is free to give up)
                    nc.tensor.matmul(
                        ps_s[:, 512 + tB:1024], kk[0:64, kiB * 128:(kiB + 1) * 128],
                        qq[0:64, qs0 + tB:qs0 + 512], start=True, stop=True)
                else:
                    nc.tensor.matmul(
                        ps_s[:, 512 + tB:1024], kk[64:128, kiB * 128:(kiB + 1) * 128],
                        qq[64:128, qs0 + tB:qs0 + 512], start=True, stop=True,
                        tile_position=(64, 0))
                ex = expp.tile([128, 1024], fp16, tag="ex", name=f"ex{jb}_{p}")
                ex_of[(jb, p)] = ex
                emit_exp(ex, ps_s, tA, tB, rA, rB)
                pend.append((jb, p))

            def emit_av(jb, p):
                npair = 2 * jb + 2
                last = 2 * npair - 1
                kiA, kiB = 2 * p, 2 * p + 1
                tA = 128 * max(kiA - 4 * jb, 0)
                tB = 128 * max(kiB - 4 * jb, 0)
                if p == 0:
                    norm_ps[jb] = psO.tile([65, 512], f32, tag="o",
                                           name=f"ps_o{jb}")
                ps_o = norm_ps[jb]
                ex = ex_of.pop((jb, p))
                nc.tensor.matmul(ps_o[:, tA:512], vaug[:, kiA, :],
                                 ex[:, tA:512], start=(2 * p == 0),
                                 stop=(2 * p == last))
                nc.tensor.matmul(ps_o[:, tB:512], vaug[:, kiB, :],
                                 ex[:, 512 + tB:1024],
                                 start=(2 * p + 1 == 0),
                                 stop=(2 * p + 1 == last))

            def flush_avs(upto_jb, finish=True):
                while pend and pend[0][0] <= upto_jb:
                    jb, p = pend.pop(0)
                    emit_av(jb, p)
                    if p == 2 * jb + 1 and finish:
                        norm_finish(jb)

            # round structure: proj(r) and the new round's first scores go
            # ahead of the previous round's AVs in the in-order PE stream,
            # so neither DMA-stalled projections nor ACT-stalled AVs ever
            # block the exp feed.
            for r in range(QB):
                proj(r)
                if r == 0:
                    proj_rest(0)       # jb0's first pairs are diagonal
                    emit_scores(0, 0)
                    emit_scores(0, 1)
                    continue
                emit_scores(r, 0)
                emit_scores(r, 1)
                flush_avs(r - 1)
                proj_rest(r)
                npair = 2 * r + 2
                for p in range(2, npair):
                    emit_scores(r, p)
                    # trail this round's AVs two pairs behind the scores
                    while len(pend) > 2:
                        jb_, p_ = pend.pop(0)
                        emit_av(jb_, p_)
                        if p_ == 2 * jb_ + 1:
                            norm_finish(jb_)
            # endgame: remaining jb3 AVs, first output half shipped before
            # the last (tA=256-trimmed) pair so the HBM-write receipt
            # overlaps it.
            while pend:
                jb_, p_ = pend.pop(0)
                emit_av(jb_, p_)
                if jb_ == QB - 1 and p_ == 2 * jb_ and len(pend) == 1:
                    norm_finish(jb_, 0, 256)
            norm_finish(QB - 1, 256, 512)

    _thin_matmul_updates(nc)
    _split_sync_waits(nc)
    nc.finalize()
    return nc


def kernel(x, Wq, Wk, Wv, attention_mask=None, **_unused):
    from concourse.bass_utils import run_bass_kernel_spmd

    if "nc" not in _cache:
        _cache["nc"] = _build_nc()
    nc = _cache["nc"]

    def chunked(wT):   # [E, 128] -> [128, E] partition-major chunk layout
        return wT.reshape(EC, 128, 128).transpose(1, 0, 2).reshape(128, E)

    wqqT = np.concatenate([np.asarray(Wq) * QSCALE, np.asarray(Wq) * QSCALE], 0).T.astype(np.float16)
    wkvT = np.concatenate([np.asarray(Wk), np.asarray(Wv)], 0).T.astype(np.float16)
    tri = (np.arange(128)[:, None] <= np.arange(128)[None, :]).astype(np.float16)
    dup = np.zeros((128, 128), np.float16)
    dup[np.arange(128) % 64, np.arange(128)] = 1.0
    wconst = np.ascontiguousarray(
        np.concatenate([chunked(wqqT), chunked(wkvT), tri, dup], 1))
    x = np.asarray(x)
    in_maps = [
        {
            "xh": np.ascontiguousarray(
                x[b].T.astype(np.float16).reshape(EC, 128, QB, 512)
                .transpose(1, 2, 0, 3).reshape(128, QB, EC * 512)),
            "wconst": wconst,
        }
        for b in range(B)
    ]
    import os

    tmpdir = None
    if os.environ.get("BASS_TRACE"):
        tmpdir = os.environ.get("BASS_TRACE_DIR", "/tmp/bass_trace")
        os.makedirs(tmpdir, exist_ok=True)
    res = run_bass_kernel_spmd(nc, in_maps, core_ids=list(range(NCORES)), tmpdir=tmpdir)
    out = np.stack(
        [(res.results[b]["out"][0:D] / res.results[b]["out"][D:D + 1]).T
         for b in range(B)], 0)
    _cache["last_exec_time_ns"] = res.exec_time_ns
    _cache["trace_dir"] = tmpdir
    return out.astype(np.float32)
